# revision 54
# baseline (speedup 1.0000x reference)
"""GAT (2-layer) on 8 NeuronCores — Bass/Tile kernel.

Strategy (dst-sharded graph parallel):
  - Each core owns 12500 destination nodes, split into 6 sub-shards
    (round-robin over the degree-sorted order) so each sub-shard's quad
    table stays within dma_gather's int16 index range.
  - Slot layout: degree-sorted 128-dst tiles, per-tile slot capacity D from
    a multiple-of-4 grid. Slots are grouped 4-at-a-time into "quads"; the
    halo table holds one 512B row per distinct quad (4 x 64 fp16 features),
    so each gather descriptor moves 512B (no sub-512B DMA penalty).
  - Launch A: per-core Wh1^T = (x W1 + b)^T + attention scalars s_i/s_j.
  - Host between launches: packs quad tables from device-computed Wh
    (fp16), expands s_j per slot (f32, with -1e30 at pad slots, which
    doubles as the softmax pad mask), folds bA into s_i. Index-only work
    plus value repacking; all model FLOPs run on device.
  - Launch B (x2, one per GAT layer): wide quad dma_gathers, segment
    softmax over the slot axis (leaky-relu on DVE, exp broadcast-expanded
    to fp16 on Act), 2x-rate fp16 multiply + in-place halving-tree slot
    sum, alpha-normalize + leaky, per-tile PE transpose into shared PSUM
    chunks, epilogue matmul with the next layer's weights.
"""

import bisect
import dataclasses
import hashlib
import numpy as np

import concourse.bacc as bacc
import concourse.tile as tile
from concourse import bass, mybir, bass_utils
from concourse.masks import make_identity

F32 = mybir.dt.float32
F16 = mybir.dt.float16
I16 = mybir.dt.int16

N_NODES = 100000
N_CORES = 8
DPC = N_NODES // N_CORES
F = 64
IN_C = 128
NSUB = 6
WQMAX = 32  # quad columns per gather call (128 slots)
SMAX = 4 * WQMAX  # slot columns per gather call
GRID = [4, 8, 12, 16, 20, 24, 28, 32, 36, 40, 44, 48, 56, 64, 80, 96, 128]
CHT = 4  # tiles per epilogue chunk (512 dsts; fp32 matmul N<=512)
NEG_BIG = -1.0e30
ALPHA = 0.2


@dataclasses.dataclass
class Schedule:
    n_tiles: int
    w_total: int  # slot columns
    wq_total: int  # quad columns
    rsubq: int  # quad-table rows per sub-shard
    tiles: list  # per global tile: (sub, D)
    calls: list  # (sub, qcol0, Wq, parts) ; parts: [(tile0, D, ntc, lq)]
    perms: list  # per core: int64 [n_tiles*128], local dst or -1
    idx16: list  # per core: int16 [128, 8*wq_total]
    qrows: list  # per core: (row_ids, nodes[R,4] int32 with -1 pads)
    edges: list  # per core: (e_p, e_col, e_src) for sj_slot expansion


def _grid_up(x):
    return GRID[bisect.bisect_left(GRID, max(1, int(x)))]


def build_schedule(edge_index: np.ndarray) -> Schedule:
    src = np.asarray(edge_index[0], dtype=np.int64)
    dst = np.asarray(edge_index[1], dtype=np.int64)
    E = src.shape[0]
    order = np.argsort(dst, kind="stable")
    src_s = src[order]
    dst_s = dst[order]
    deg_all = np.bincount(dst, minlength=N_NODES)
    starts_all = np.concatenate([[0], np.cumsum(deg_all)])
    k_s = np.arange(E) - starts_all[dst_s]

    # ascending-degree round robin; the LAST sub-shard is reversed
    # (descending) so the program's final tiles are small and the epilogue
    # tail is short
    core_subs = []
    for c in range(N_CORES):
        deg = deg_all[c * DPC : (c + 1) * DPC]
        rank = np.argsort(deg, kind="stable")
        subs = [rank[s::NSUB] for s in range(NSUB)]
        subs[NSUB - 1] = subs[NSUB - 1][::-1]
        core_subs.append(subs)

    def _blockify_s(s, lst, nt):
        # partial tile holds the low-degree end: front-pad ascending subs,
        # back-pad the reversed (descending) last sub
        block = np.full(nt * 128, -1, np.int64)
        if s == NSUB - 1:
            block[: len(lst)] = lst
        else:
            block[nt * 128 - len(lst) :] = lst
        return block

    tiles = []
    sub_nt = []
    for s in range(NSUB):
        nt = max(-(-len(core_subs[c][s]) // 128) for c in range(N_CORES))
        sub_nt.append(nt)
        tmax = np.zeros(nt)
        for c in range(N_CORES):
            block = _blockify_s(s, core_subs[c][s], nt)
            d = np.where(
                block >= 0, deg_all[np.clip(c * DPC + block, 0, N_NODES - 1)], 0
            )
            tmax = np.maximum(tmax, d.reshape(nt, 128).max(1))
        for t in range(nt):
            tiles.append((s, _grid_up(tmax[t])))
    n_tiles = len(tiles)

    # runs of equal (sub, D) packed into gather calls of <= WQMAX quad cols
    runs = []
    i = 0
    while i < n_tiles:
        s, D = tiles[i]
        n = 1
        while i + n < n_tiles and tiles[i + n] == (s, D):
            n += 1
        runs.append((s, i, D, n))
        i += n
    # pack runs into calls by SLOT width; a call's quad width is its slot
    # width rounded up to a multiple of 4 (dead pad slots at the call end)
    calls = []
    cur_s, cur, cw = None, [], 0
    for (s, t0, D, n) in runs:
        rem_t0, rem_n = t0, n
        while rem_n:
            lim = 32 if not calls else SMAX  # small first call: shorter ramp
            lim = max(lim, D)
            if cur and (cur_s != s or cw + D > lim):
                calls.append((cur_s, 0, -(-cw // 4), cur))
                cur_s, cur, cw = None, [], 0
                continue
            take = min(rem_n, (lim - cw) // D)
            assert take > 0, (s, D, n, cw, lim)
            cur_s = s
            cur.append((rem_t0, D, take, cw))
            cw += D * take
            rem_t0 += take
            rem_n -= take
    if cur:
        calls.append((cur_s, 0, -(-cw // 4), cur))
    # split a small tail off the last call so the drain is short
    s_l, _, _, parts_l = calls[-1]
    tot_l = sum(D * ntc for (_, D, ntc, _) in parts_l)
    if tot_l > 32:
        target = tot_l - 16
        p1, p2, acc, w1, w2 = [], [], 0, 0, 0
        for (t0, D, ntc, lc) in parts_l:
            for tl in range(ntc):
                if acc < target:
                    if p1 and p1[-1][0] + p1[-1][2] == t0 + tl and p1[-1][1] == D:
                        p1[-1] = (p1[-1][0], D, p1[-1][2] + 1, p1[-1][3])
                    else:
                        p1.append((t0 + tl, D, 1, w1))
                    w1 += D
                else:
                    if p2 and p2[-1][0] + p2[-1][2] == t0 + tl and p2[-1][1] == D:
                        p2[-1] = (p2[-1][0], D, p2[-1][2] + 1, p2[-1][3])
                    else:
                        p2.append((t0 + tl, D, 1, w2))
                    w2 += D
                acc += D
        p1 = [tuple(x) for x in p1]
        p2 = [tuple(x) for x in p2]
        calls[-1] = (s_l, 0, -(-w1 // 4), p1)
        calls.append((s_l, 0, -(-w2 // 4), p2))
    qcol = 0
    for j, (s, _, Wq, parts) in enumerate(calls):
        calls[j] = (s, qcol, Wq, parts)
        qcol += Wq
    wq_total = qcol
    w_total = 4 * wq_total
    tile_col0 = np.zeros(n_tiles, np.int64)
    for (s, qcol0, Wq, parts) in calls:
        for (t0, D, ntc, lc) in parts:
            for tl in range(ntc):
                tile_col0[t0 + tl] = 4 * qcol0 + lc + tl * D
    sub_of_tile = np.array([s for (s, D) in tiles], np.int64)

    perms, idx16s, qrowss, edgess = [], [], [], []
    rsub_max = 0
    percore = []
    for c in range(N_CORES):
        perm = np.full(n_tiles * 128, -1, dtype=np.int64)
        ti = 0
        for s in range(NSUB):
            nt = sub_nt[s]
            perm[ti * 128 : (ti + nt) * 128] = _blockify_s(s, core_subs[c][s], nt)
            ti += nt

        real = perm >= 0
        pos_of_dst = np.empty(DPC, np.int64)
        pos_of_dst[perm[real]] = np.flatnonzero(real)
        gtile_of_dst = pos_of_dst // 128
        p_of_dst = pos_of_dst % 128

        lo, hi = starts_all[c * DPC], starts_all[(c + 1) * DPC]
        e_src = src_s[lo:hi].astype(np.int32)
        e_dstl = dst_s[lo:hi] - c * DPC
        e_k = k_s[lo:hi]
        e_tile = gtile_of_dst[e_dstl]
        e_p = p_of_dst[e_dstl].astype(np.int32)
        e_col = (tile_col0[e_tile] + e_k).astype(np.int32)

        # slot-level source matrix (-1 = pad), then quads + per-sub dedup
        S = np.full((128, w_total), -1, np.int32)
        S[e_p, e_col] = e_src
        idxq = np.zeros((128, wq_total), np.int32)
        qrows_l, qnodes_l = [], []
        for s in range(NSUB):
            qsel = [
                (qcol0, Wq)
                for (ss, qcol0, Wq, parts) in calls
                if ss == s
            ]
            cols = np.concatenate(
                [np.arange(q0, q0 + Wq) for (q0, Wq) in qsel]
            )
            quads = S[:, (4 * cols[:, None] + np.arange(4)).reshape(-1)]
            quads = quads.reshape(128, len(cols), 4)
            flat = np.ascontiguousarray(quads.reshape(-1, 4))
            u, inv = np.unique(flat.view("V16").ravel(), return_inverse=True)
            nu = len(u)
            rsub_max = max(rsub_max, nu)
            uq = u.view(np.int32).reshape(-1, 4)
            idxq[:, cols] = inv.reshape(128, len(cols))
            qrows_l.append(uq)
        percore.append((perm, idxq, qrows_l, (e_p, e_col, e_src)))

    rsubq = -(-int(rsub_max) // 128) * 128
    for c in range(N_CORES):
        perm, idxq, qrows_l, edges = percore[c]
        idx16 = np.zeros((128, 8 * wq_total), np.int16)
        for (s, qcol0, Wq, parts) in calls:
            flat = idxq[:, qcol0 : qcol0 + Wq].T.ravel()
            idx16[:, 8 * qcol0 : 8 * (qcol0 + Wq)] = np.tile(
                flat.reshape(-1, 16).T, (8, 1)
            ).astype(np.int16)
        rows = np.concatenate(
            [s * rsubq + np.arange(len(qrows_l[s])) for s in range(NSUB)]
        )
        nodes = np.concatenate(qrows_l, axis=0)
        perms.append(perm)
        idx16s.append(idx16)
        qrowss.append((rows, nodes))
        edgess.append(edges)

    return Schedule(
        n_tiles,
        w_total,
        wq_total,
        rsubq,
        tiles,
        calls,
        perms,
        idx16s,
        qrowss,
        edgess,
    )


# ---------------------------------------------------------------- prog A
def build_progA(n_loc=DPC, in_c=IN_C, f=F):
    nc = bacc.Bacc("TRN2", target_bir_lowering=False, debug=False, num_devices=N_CORES)
    xT = nc.dram_tensor("xT", [in_c, n_loc], F16, kind="ExternalInput").ap()
    W = nc.dram_tensor("W", [in_c, f], F16, kind="ExternalInput").ap()
    bW = nc.dram_tensor("bW", [f, 1], F32, kind="ExternalInput").ap()
    WA = nc.dram_tensor("WA", [in_c, 2], F16, kind="ExternalInput").ap()
    bA2 = nc.dram_tensor("bA2", [2, 1], F32, kind="ExternalInput").ap()
    whT = nc.dram_tensor("whT", [f, n_loc], F32, kind="ExternalOutput").ap()
    s = nc.dram_tensor("s", [2, n_loc], F32, kind="ExternalOutput").ap()

    with tile.TileContext(nc) as tc:
        with tc.tile_pool(name="sb", bufs=1) as pool, tc.tile_pool(
            name="ps", bufs=4, space="PSUM"
        ) as pps, tc.tile_pool(name="sb2", bufs=2) as pool2:
            W_sb = pool.tile([in_c, f], F16)
            nc.sync.dma_start(out=W_sb[:], in_=W[:, :])
            bW_sb = pool.tile([f, 1], F32)
            nc.sync.dma_start(out=bW_sb[:], in_=bW[:, :])
            WA_sb = pool.tile([in_c, 2], F16)
            nc.sync.dma_start(out=WA_sb[:], in_=WA[:, :])
            bA2_sb = pool.tile([2, 1], F32)
            nc.sync.dma_start(out=bA2_sb[:], in_=bA2[:, :])
            xT_sb = pool.tile([in_c, n_loc], F16)
            XCH = 3125
            for x0 in range(0, n_loc, XCH):
                xc = min(XCH, n_loc - x0)
                nc.sync.dma_start(
                    out=xT_sb[:, x0 : x0 + xc], in_=xT[:, x0 : x0 + xc]
                )

            CH = 512
            GRP = 4  # store in 2048-column groups
            wh_g = None
            s_g = None
            for c0 in range(0, n_loc, CH):
                ch = min(CH, n_loc - c0)
                gi = (c0 // CH) % GRP
                if gi == 0:
                    wh_g = pool2.tile([f, GRP * CH], F32, tag="whg")
                    s_g = pool2.tile([2, GRP * CH], F32, tag="sg")
                ps_w = pps.tile([f, CH], F32, space="PSUM")
                nc.tensor.matmul(
                    out=ps_w[:, :ch],
                    lhsT=W_sb[:],
                    rhs=xT_sb[:, c0 : c0 + ch],
                    start=True,
                    stop=True,
                )
                nc.scalar.activation(
                    out=wh_g[:, gi * CH : gi * CH + ch],
                    in_=ps_w[:, :ch],
                    func=mybir.ActivationFunctionType.Identity,
                    bias=bW_sb[:],
                )
                ps_s = pps.tile([2, CH], F32, space="PSUM")
                nc.tensor.matmul(
                    out=ps_s[:, :ch],
                    lhsT=WA_sb[:],
                    rhs=xT_sb[:, c0 : c0 + ch],
                    start=True,
                    stop=True,
                )
                nc.vector.tensor_scalar(
                    out=s_g[:, gi * CH : gi * CH + ch],
                    in0=ps_s[:, :ch],
                    scalar1=bA2_sb[:, 0:1],
                    scalar2=None,
                    op0=mybir.AluOpType.add,
                )
                if gi == GRP - 1 or c0 + ch >= n_loc:
                    g0 = (c0 // CH // GRP) * GRP * CH
                    gl = c0 + ch - g0
                    nc.sync.dma_start(
                        out=whT[:, g0 : g0 + gl], in_=wh_g[:, :gl]
                    )
                    nc.sync.dma_start(out=s[:, g0 : g0 + gl], in_=s_g[:, :gl])
    nc.compile()
    return nc


# ---------------------------------------------------------------- prog B
def build_progB(sched: Schedule, f=F):
    NT = sched.n_tiles
    WTOT = sched.w_total
    WQTOT = sched.wq_total
    RSUBQ = sched.rsubq
    nc = bacc.Bacc("TRN2", target_bir_lowering=False, debug=False, num_devices=N_CORES)
    tableq = nc.dram_tensor(
        "tableq", [NSUB * RSUBQ, 2 * f], F32, kind="ExternalInput"
    ).ap()
    idx_d = nc.dram_tensor("idx", [128, 8 * WQTOT], I16, kind="ExternalInput").ap()
    sj_d = nc.dram_tensor("sj", [128, WTOT], F32, kind="ExternalInput").ap()
    si_d = nc.dram_tensor("si", [128, NT], F32, kind="ExternalInput").ap()
    # packed small consts: cols 0-63 Wn, 64 bWn, 65-66 As
    wp_d = nc.dram_tensor("wpack", [f, f + 4], F32, kind="ExternalInput").ap()
    whnT = nc.dram_tensor("whnT", [f, NT * 128], F32, kind="ExternalOutput").ap()
    sn = nc.dram_tensor("sn", [2, NT * 128], F32, kind="ExternalOutput").ap()

    X = mybir.AxisListType.X
    AF = mybir.ActivationFunctionType
    OP = mybir.AluOpType

    def v(ap, dims, off=0):
        return dataclasses.replace(
            ap,
            ap=[list(ap.ap[0])] + [list(d) for d in dims],
            offset=ap.offset + off,
        )

    nq = min(4, nc.num_swdge_queues)
    NTCMAX = max(ntc for (_, _, _, parts) in sched.calls for (_, _, ntc, _) in parts)

    with tile.TileContext(nc) as tc:
        with tc.tile_pool(name="const", bufs=1) as pc, tc.tile_pool(
            name="gat", bufs=3
        ) as pg, tc.tile_pool(name="exw", bufs=3) as px, tc.tile_pool(
            name="work", bufs=3
        ) as pw, tc.tile_pool(name="ht", bufs=2) as ph, tc.tile_pool(
            name="ps", bufs=2, space="PSUM"
        ) as pps, tc.tile_pool(name="ep", bufs=2) as pep:
            si_sb = pc.tile([128, NT], F32)
            nc.sync.dma_start(out=si_sb[:], in_=si_d[:, :])
            sj_sb = pc.tile([128, WTOT], F32)
            idx_sb = pc.tile([128, 8 * WQTOT], I16)
            wp_sb = pc.tile([f, f + 4], F32)
            Wn_sb = wp_sb[:, :f]
            bWn_sb = wp_sb[:, f : f + 1]
            As_sb = wp_sb[:, f + 1 : f + 3]
            ident = pc.tile([128, 128], F32)

            def emit_consts():
                # deferred past the first call's gather so the startup HWDGE
                # FIFO isn't serialized ahead of it
                nc.sync.dma_start(out=wp_sb[:], in_=wp_d[:, :])
                make_identity(nc, ident[:])

            ps_ch = None

            def flush_chunk(ck, ntl):
                cols = ntl * 128
                hTL = pep.tile([f, CHT * 128], F32, tag="hTL")
                nc.scalar.activation(
                    out=hTL[:, :cols],
                    in_=ps_ch[:, :cols],
                    func=AF.Identity,
                )
                ps_w = pps.tile([f, CHT * 128], F32, tag="psw", space="PSUM")
                nc.tensor.matmul(
                    out=ps_w[:, :cols],
                    lhsT=Wn_sb[:],
                    rhs=hTL[:, :cols],
                    start=True,
                    stop=True,
                )
                whn_sb = pep.tile([f, CHT * 128], F32, tag="whn")
                nc.scalar.activation(
                    out=whn_sb[:, :cols],
                    in_=ps_w[:, :cols],
                    func=AF.Identity,
                    bias=bWn_sb[:],
                )
                nc.sync.dma_start(
                    out=whnT[:, ck * CHT * 128 : ck * CHT * 128 + cols],
                    in_=whn_sb[:, :cols],
                )
                ps_s = pps.tile([2, CHT * 128], F32, tag="pss", space="PSUM")
                nc.tensor.matmul(
                    out=ps_s[:, :cols],
                    lhsT=As_sb,
                    rhs=whn_sb[:, :cols],
                    start=True,
                    stop=True,
                )
                s_sb = pep.tile([2, CHT * 128], F32, tag="ssb")
                nc.scalar.activation(
                    out=s_sb[:, :cols], in_=ps_s[:, :cols], func=AF.Identity
                )
                nc.sync.dma_start(
                    out=sn[:, ck * CHT * 128 : ck * CHT * 128 + cols],
                    in_=s_sb[:, :cols],
                )

            gq = 0

            def stage1(ci):
                nonlocal gq
                s, qcol0, Wq, parts = sched.calls[ci]
                # per-call slices of the idx / sj constants (shorter ramp)
                nc.sync.dma_start(
                    out=idx_sb[:, 8 * qcol0 : 8 * (qcol0 + Wq)],
                    in_=idx_d[:, 8 * qcol0 : 8 * (qcol0 + Wq)],
                )
                nc.sync.dma_start(
                    out=sj_sb[:, 4 * qcol0 : 4 * (qcol0 + Wq)],
                    in_=sj_d[:, 4 * qcol0 : 4 * (qcol0 + Wq)],
                )
                gbuf = pg.tile([128, WQMAX * 2 * f], F32, tag="gbuf")
                # hw limit: <=1024 indices per dma_gather -> <=8 quad columns
                for j0 in range(0, Wq, 8):
                    jw = min(8, Wq - j0)
                    nc.gpsimd.dma_gather(
                        out_ap=v(
                            gbuf[:], [(2 * f, jw), (1, 2 * f)], off=j0 * 2 * f
                        ),
                        in_ap=tableq[s * RSUBQ : (s + 1) * RSUBQ, :],
                        idxs_ap=idx_sb[:, 8 * (qcol0 + j0) : 8 * (qcol0 + j0 + jw)],
                        num_idxs=128 * jw,
                        num_idxs_reg=128 * jw,
                        elem_size=2 * f,
                        queue_num=gq % nq,
                    )
                    gq += 1
                gbuf16 = gbuf[:].bitcast(F16)  # slot i feats at f16 cols [64i,+64)
                exw = px.tile([128, WQMAX * 4 * f], F16, tag="exw")

                # attention logits + exp for every part
                for (t0, D, ntc, lc) in parts:
                    Wr = D * ntc
                    ls = lc  # slot offset within call
                    sc = 4 * qcol0 + lc  # global slot column
                    # e_pre = sj + si'  (si' = si + bA; sj = NEG_BIG at pads)
                    epre = pw.tile([128, 4 * WQMAX], F32, tag="epre")
                    nc.vector.tensor_tensor(
                        out=v(epre[:], [(D, ntc), (1, D)]),
                        in0=v(sj_sb[:], [(D, ntc), (1, D)], off=sc),
                        in1=si_sb[:, t0 : t0 + ntc].to_broadcast([128, ntc, D]),
                        op=OP.add,
                    )
                    # e = leaky_relu(e_pre)  (DVE: alpha*x then max)
                    e1 = pw.tile([128, 4 * WQMAX], F32, tag="e1")
                    nc.vector.tensor_scalar(
                        out=e1[:, :Wr],
                        in0=epre[:, :Wr],
                        scalar1=ALPHA,
                        scalar2=None,
                        op0=OP.mult,
                    )
                    nc.vector.tensor_tensor(
                        out=e1[:, :Wr], in0=e1[:, :Wr], in1=epre[:, :Wr], op=OP.max
                    )
                    # segment softmax over the slot axis
                    m = pw.tile([128, NTCMAX], F32, tag="m")
                    nc.vector.tensor_reduce(
                        out=m[:, :ntc],
                        in_=v(e1[:], [(D, ntc), (1, D)]),
                        axis=X,
                        op=OP.max,
                    )
                    nc.vector.tensor_tensor(
                        out=v(e1[:], [(D, ntc), (1, D)]),
                        in0=v(e1[:], [(D, ntc), (1, D)]),
                        in1=m[:, :ntc].to_broadcast([128, ntc, D]),
                        op=OP.subtract,
                    )
                    # exp, broadcast-expanded across the feature axis (fp16)
                    nc.scalar.activation(
                        out=v(exw[:], [(f * D, ntc), (f, D), (1, f)], off=ls * f),
                        in_=v(e1[:], [(D, ntc), (1, D), (0, f)]),
                        func=AF.Exp,
                    )
                return gbuf16, exw

            def stage2(ci, gbuf16, exw):
                nonlocal ps_ch
                s, qcol0, Wq, parts = sched.calls[ci]
                # denominator, weighted message sum, epilogue
                for (t0, D, ntc, lc) in parts:
                    Wr = D * ntc
                    ls = lc
                    den = pw.tile([128, NTCMAX], F32, tag="den")
                    nc.vector.tensor_reduce(
                        out=den[:, :ntc],
                        in_=v(exw[:], [(f * D, ntc), (f, D)], off=ls * f),
                        axis=X,
                        op=OP.add,
                    )
                    rden = pw.tile([128, NTCMAX], F32, tag="rden")
                    nc.vector.reciprocal(out=rden[:, :ntc], in_=den[:, :ntc])
                    # weighted messages: exw *= Wh16 (in place, 2x fp16)
                    nc.vector.tensor_tensor(
                        out=v(exw[:], [(f * D, ntc), (f, D), (1, f)], off=ls * f),
                        in0=v(gbuf16, [(f * D, ntc), (f, D), (1, f)], off=ls * f),
                        in1=v(exw[:], [(f * D, ntc), (f, D), (1, f)], off=ls * f),
                        op=OP.mult,
                    )
                    # halving-tree sum over slots -> slot 0 of each tile
                    cur = D
                    while cur > 1:
                        if cur % 2:
                            nc.vector.tensor_tensor(
                                out=v(exw[:], [(f * D, ntc), (1, f)], off=ls * f),
                                in0=v(exw[:], [(f * D, ntc), (1, f)], off=ls * f),
                                in1=v(
                                    exw[:],
                                    [(f * D, ntc), (1, f)],
                                    off=(ls + cur - 1) * f,
                                ),
                                op=OP.add,
                            )
                            cur -= 1
                        h = cur // 2
                        nc.vector.tensor_tensor(
                            out=v(exw[:], [(f * D, ntc), (f, h), (1, f)], off=ls * f),
                            in0=v(exw[:], [(f * D, ntc), (f, h), (1, f)], off=ls * f),
                            in1=v(
                                exw[:],
                                [(f * D, ntc), (f, h), (1, f)],
                                off=(ls + h) * f,
                            ),
                            op=OP.add,
                        )
                        cur = h
                    # normalize + leaky (dst-major), then transpose per tile
                    ht = ph.tile([128, NTCMAX * f], F32, tag="ht")
                    nc.vector.tensor_tensor(
                        out=v(ht[:], [(f, ntc), (1, f)]),
                        in0=v(exw[:], [(f * D, ntc), (1, f)], off=ls * f),
                        in1=rden[:, :ntc].to_broadcast([128, ntc, f]),
                        op=OP.mult,
                    )
                    ht2 = ph.tile([128, NTCMAX * f], F32, tag="ht2")
                    nc.vector.tensor_scalar(
                        out=ht2[:, : ntc * f],
                        in0=ht[:, : ntc * f],
                        scalar1=ALPHA,
                        scalar2=None,
                        op0=OP.mult,
                    )
                    nc.vector.tensor_tensor(
                        out=ht[:, : ntc * f],
                        in0=ht[:, : ntc * f],
                        in1=ht2[:, : ntc * f],
                        op=OP.max,
                    )
                    for tl in range(ntc):
                        t = t0 + tl
                        j = t % CHT
                        if j == 0:
                            ps_ch = pps.tile(
                                [f, CHT * 128], F32, tag="psch", space="PSUM"
                            )
                        nc.tensor.transpose(
                            out=ps_ch[:, j * 128 : (j + 1) * 128],
                            in_=ht[:, tl * f : (tl + 1) * f],
                            identity=ident[:],
                        )
                        if j == CHT - 1 or t == NT - 1:
                            flush_chunk(t // CHT, j + 1)

            # software pipeline: stage1 of call N+1 is emitted before stage2
            # of call N so the DVE never stalls on Act's exp at call
            # boundaries
            prev = None
            for ci in range(len(sched.calls)):
                ctx = stage1(ci)
                if ci == 0:
                    emit_consts()
                if prev is not None:
                    stage2(prev[0], *prev[1])
                prev = (ci, ctx)
            stage2(prev[0], *prev[1])
    nc.compile()
    return nc


# ---------------------------------------------------------------- driver
_cache = {}
TRACE = False
LAST_HW_NS = []
LAST_RESULTS = []


def _run(nc, in_maps, cores):
    res = bass_utils.run_bass_kernel_spmd(nc, in_maps, core_ids=cores, trace=TRACE)
    if TRACE:
        LAST_RESULTS.append(res)
        if res.exec_time_ns:
            LAST_HW_NS.append(res.exec_time_ns)
    return res


def _get_schedule(edge_index):
    fp = hashlib.sha1(np.ascontiguousarray(edge_index)).hexdigest()
    key = ("sched", fp)
    if key not in _cache:
        _cache[key] = build_schedule(edge_index)
    return _cache[key]


def _pack_table(sched, c, wh16, sj_full):
    rows, nodes = sched.qrows[c]
    table = np.zeros((NSUB * sched.rsubq, 2 * F), np.float32)
    blk = wh16[np.clip(nodes, 0, N_NODES - 1)]
    blk[nodes < 0] = 0
    table[rows] = blk.reshape(len(rows), 4 * F).view(np.float32)
    return table


def kernel(x, edge_index, W1, bW1, A1, bA1, W2, bW2, A2, bA2, Wfc, bfc):
    x = np.asarray(x, dtype=np.float32)
    edge_index = np.asarray(edge_index)
    W1 = np.asarray(W1, np.float32)
    bW1 = np.asarray(bW1, np.float32)
    A1 = np.asarray(A1, np.float32)
    bA1 = np.asarray(bA1, np.float32)
    W2 = np.asarray(W2, np.float32)
    bW2 = np.asarray(bW2, np.float32)
    A2 = np.asarray(A2, np.float32)
    bA2 = np.asarray(bA2, np.float32)
    Wfc = np.asarray(Wfc, np.float32)
    bfc = np.asarray(bfc, np.float32)

    sched = _get_schedule(edge_index)
    cores = list(range(N_CORES))

    if "A" not in _cache:
        _cache["A"] = build_progA()
    ncA = _cache["A"]
    inA = []
    x16 = x.astype(np.float16)
    A1cat = np.concatenate([A1[:F], A1[F:]], axis=1)  # [64, 2]
    WA = (W1 @ A1cat).astype(np.float16)  # [128, 2]
    bA2v = (bW1 @ A1cat).reshape(2, 1).astype(np.float32)
    for c in cores:
        xT = np.ascontiguousarray(x16[c * DPC : (c + 1) * DPC].T)
        inA.append(
            {
                "xT": xT,
                "W": W1.astype(np.float16),
                "bW": bW1.reshape(F, 1),
                "WA": WA,
                "bA2": bA2v,
            }
        )
    LAST_HW_NS.clear()
    LAST_RESULTS.clear()
    resA = _run(ncA, inA, cores)
    wh = np.concatenate([resA.results[c]["whT"].T for c in cores], axis=0)
    s_all = np.concatenate([resA.results[c]["s"] for c in cores], axis=1)
    si_full, sj_full = s_all[0], s_all[1]

    key = ("B", sched.n_tiles, sched.wq_total, sched.rsubq, tuple(sched.tiles))
    if key not in _cache:
        _cache[key] = build_progB(sched)
    ncB = _cache[key]

    def launch_B(wh_full, si_f, sj_f, bA, Wn, bWn, An):
        bA0 = np.float32(bA.reshape(-1)[0])
        wh16 = wh_full.astype(np.float16)
        wpack = np.zeros((F, F + 4), np.float32)
        wpack[:, :F] = Wn
        wpack[:, F] = bWn.reshape(F)
        wpack[:, F + 1 : F + 3] = An
        inB = []
        for c in cores:
            perm = sched.perms[c]
            real = perm >= 0
            gids = c * DPC + perm[real]
            tmp = np.zeros(sched.n_tiles * 128, np.float32)
            tmp[real] = si_f[gids] + bA0
            si_arr = np.ascontiguousarray(tmp.reshape(sched.n_tiles, 128).T)
            e_p, e_col, e_src = sched.edges[c]
            sj_arr = np.full((128, sched.w_total), np.float32(NEG_BIG))
            sj_arr[e_p, e_col] = sj_f[e_src]
            inB.append(
                {
                    "tableq": _pack_table(sched, c, wh16, sj_f),
                    "idx": sched.idx16[c],
                    "sj": sj_arr,
                    "si": si_arr,
                    "wpack": wpack,
                }
            )
        res = _run(ncB, inB, cores)
        whn = np.zeros((N_NODES, F), np.float32)
        sn_i = np.zeros(N_NODES, np.float32)
        sn_j = np.zeros(N_NODES, np.float32)
        for c in cores:
            perm = sched.perms[c]
            real = perm >= 0
            gids = c * DPC + perm[real]
            whn[gids] = res.results[c]["whnT"].T[real]
            sn_c = res.results[c]["sn"]
            sn_i[gids] = sn_c[0][real]
            sn_j[gids] = sn_c[1][real]
        return whn, sn_i, sn_j

    As2 = np.ascontiguousarray(np.concatenate([A2[:F], A2[F:]], axis=1))
    wh2, si2, sj2 = launch_B(wh, si_full, sj_full, bA1, W2, bW2, As2)
    out, _, _ = launch_B(wh2, si2, sj2, bA2, Wfc, bfc, np.zeros((F, 2), np.float32))
    return out.astype(np.float32)


# revision 60
# speedup vs baseline: 1.0364x; 1.0364x over previous
"""GAT (2-layer) on 8 NeuronCores — Bass/Tile kernel.

Strategy (dst-sharded graph parallel):
  - Each core owns 12500 destination nodes, split into 6 sub-shards
    (round-robin over the degree-sorted order) so each sub-shard's quad
    table stays within dma_gather's int16 index range.
  - Slot layout: degree-sorted 128-dst tiles, per-tile slot capacity D from
    a multiple-of-4 grid. Slots are grouped 4-at-a-time into "quads"; the
    halo table holds one 512B row per distinct quad (4 x 64 fp16 features),
    so each gather descriptor moves 512B (no sub-512B DMA penalty).
  - Launch A: per-core Wh1^T = (x W1 + b)^T + attention scalars s_i/s_j.
  - Host between launches: packs quad tables from device-computed Wh
    (fp16), expands s_j per slot (f32, with -1e30 at pad slots, which
    doubles as the softmax pad mask), folds bA into s_i. Index-only work
    plus value repacking; all model FLOPs run on device.
  - Launch B (x2, one per GAT layer): wide quad dma_gathers, segment
    softmax over the slot axis (leaky-relu on DVE, exp broadcast-expanded
    to fp16 on Act), 2x-rate fp16 multiply + in-place halving-tree slot
    sum, alpha-normalize + leaky, per-tile PE transpose into shared PSUM
    chunks, epilogue matmul with the next layer's weights.
"""

import bisect
import dataclasses
import hashlib
import numpy as np

import concourse.bacc as bacc
import concourse.tile as tile
from concourse import bass, mybir, bass_utils
from concourse.masks import make_identity

F32 = mybir.dt.float32
F16 = mybir.dt.float16
I16 = mybir.dt.int16

N_NODES = 100000
N_CORES = 8
DPC = N_NODES // N_CORES
F = 64
IN_C = 128
NSUB = 6
WQMAX = 32  # quad columns per gather call (128 slots)
SMAX = 4 * WQMAX  # slot columns per gather call
GRID = [4, 8, 12, 16, 20, 24, 28, 32, 36, 40, 44, 48, 56, 64, 80, 96, 128]
CHT = 4  # tiles per epilogue chunk (512 dsts; fp32 matmul N<=512)
NEG_BIG = -1.0e30
ALPHA = 0.2


@dataclasses.dataclass
class Schedule:
    n_tiles: int
    w_total: int  # slot columns
    wq_total: int  # quad columns
    rsubq: int  # quad-table rows per sub-shard
    tiles: list  # per global tile: (sub, D)
    calls: list  # (sub, qcol0, Wq, parts) ; parts: [(tile0, D, ntc, lq)]
    perms: list  # per core: int64 [n_tiles*128], local dst or -1
    idx16: list  # per core: int16 [128, 8*wq_total]
    qrows: list  # per core: (row_ids, nodes[R,4] int32 with -1 pads)
    edges: list  # per core: (e_p, e_col, e_src) for sj_slot expansion


def _grid_up(x):
    return GRID[bisect.bisect_left(GRID, max(1, int(x)))]


def build_schedule(edge_index: np.ndarray) -> Schedule:
    src = np.asarray(edge_index[0], dtype=np.int64)
    dst = np.asarray(edge_index[1], dtype=np.int64)
    E = src.shape[0]
    order = np.argsort(dst, kind="stable")
    src_s = src[order]
    dst_s = dst[order]
    deg_all = np.bincount(dst, minlength=N_NODES)
    starts_all = np.concatenate([[0], np.cumsum(deg_all)])
    k_s = np.arange(E) - starts_all[dst_s]

    # ascending-degree round robin; the LAST sub-shard is reversed
    # (descending) so the program's final tiles are small and the epilogue
    # tail is short
    core_subs = []
    for c in range(N_CORES):
        deg = deg_all[c * DPC : (c + 1) * DPC]
        rank = np.argsort(deg, kind="stable")
        subs = [rank[s::NSUB] for s in range(NSUB)]
        subs[NSUB - 1] = subs[NSUB - 1][::-1]
        core_subs.append(subs)

    def _blockify_s(s, lst, nt):
        # partial tile holds the low-degree end: front-pad ascending subs,
        # back-pad the reversed (descending) last sub
        block = np.full(nt * 128, -1, np.int64)
        if s == NSUB - 1:
            block[: len(lst)] = lst
        else:
            block[nt * 128 - len(lst) :] = lst
        return block

    tiles = []
    sub_nt = []
    for s in range(NSUB):
        nt = max(-(-len(core_subs[c][s]) // 128) for c in range(N_CORES))
        sub_nt.append(nt)
        tmax = np.zeros(nt)
        for c in range(N_CORES):
            block = _blockify_s(s, core_subs[c][s], nt)
            d = np.where(
                block >= 0, deg_all[np.clip(c * DPC + block, 0, N_NODES - 1)], 0
            )
            tmax = np.maximum(tmax, d.reshape(nt, 128).max(1))
        for t in range(nt):
            tiles.append((s, _grid_up(tmax[t])))
    n_tiles = len(tiles)

    # runs of equal (sub, D) packed into gather calls of <= WQMAX quad cols
    runs = []
    i = 0
    while i < n_tiles:
        s, D = tiles[i]
        n = 1
        while i + n < n_tiles and tiles[i + n] == (s, D):
            n += 1
        runs.append((s, i, D, n))
        i += n
    # pack runs into calls by SLOT width; a call's quad width is its slot
    # width rounded up to a multiple of 4 (dead pad slots at the call end)
    calls = []
    cur_s, cur, cw = None, [], 0
    for (s, t0, D, n) in runs:
        rem_t0, rem_n = t0, n
        while rem_n:
            lim = 32 if not calls else SMAX  # small first call: shorter ramp
            lim = max(lim, D)
            if cur and (cur_s != s or cw + D > lim):
                calls.append((cur_s, 0, -(-cw // 4), cur))
                cur_s, cur, cw = None, [], 0
                continue
            take = min(rem_n, (lim - cw) // D)
            assert take > 0, (s, D, n, cw, lim)
            cur_s = s
            cur.append((rem_t0, D, take, cw))
            cw += D * take
            rem_t0 += take
            rem_n -= take
    if cur:
        calls.append((cur_s, 0, -(-cw // 4), cur))
    # split a small tail off the last call so the drain is short
    s_l, _, _, parts_l = calls[-1]
    tot_l = sum(D * ntc for (_, D, ntc, _) in parts_l)
    if tot_l > 32:
        target = tot_l - 16
        p1, p2, acc, w1, w2 = [], [], 0, 0, 0
        for (t0, D, ntc, lc) in parts_l:
            for tl in range(ntc):
                if acc < target:
                    if p1 and p1[-1][0] + p1[-1][2] == t0 + tl and p1[-1][1] == D:
                        p1[-1] = (p1[-1][0], D, p1[-1][2] + 1, p1[-1][3])
                    else:
                        p1.append((t0 + tl, D, 1, w1))
                    w1 += D
                else:
                    if p2 and p2[-1][0] + p2[-1][2] == t0 + tl and p2[-1][1] == D:
                        p2[-1] = (p2[-1][0], D, p2[-1][2] + 1, p2[-1][3])
                    else:
                        p2.append((t0 + tl, D, 1, w2))
                    w2 += D
                acc += D
        p1 = [tuple(x) for x in p1]
        p2 = [tuple(x) for x in p2]
        calls[-1] = (s_l, 0, -(-w1 // 4), p1)
        calls.append((s_l, 0, -(-w2 // 4), p2))
    qcol = 0
    for j, (s, _, Wq, parts) in enumerate(calls):
        calls[j] = (s, qcol, Wq, parts)
        qcol += Wq
    wq_total = qcol
    w_total = 4 * wq_total
    tile_col0 = np.zeros(n_tiles, np.int64)
    for (s, qcol0, Wq, parts) in calls:
        for (t0, D, ntc, lc) in parts:
            for tl in range(ntc):
                tile_col0[t0 + tl] = 4 * qcol0 + lc + tl * D
    sub_of_tile = np.array([s for (s, D) in tiles], np.int64)

    perms, idx16s, qrowss, edgess = [], [], [], []
    rsub_max = 0
    percore = []
    for c in range(N_CORES):
        perm = np.full(n_tiles * 128, -1, dtype=np.int64)
        ti = 0
        for s in range(NSUB):
            nt = sub_nt[s]
            perm[ti * 128 : (ti + nt) * 128] = _blockify_s(s, core_subs[c][s], nt)
            ti += nt

        real = perm >= 0
        pos_of_dst = np.empty(DPC, np.int64)
        pos_of_dst[perm[real]] = np.flatnonzero(real)
        gtile_of_dst = pos_of_dst // 128
        p_of_dst = pos_of_dst % 128

        lo, hi = starts_all[c * DPC], starts_all[(c + 1) * DPC]
        e_src = src_s[lo:hi].astype(np.int32)
        e_dstl = dst_s[lo:hi] - c * DPC
        e_k = k_s[lo:hi]
        e_tile = gtile_of_dst[e_dstl]
        e_p = p_of_dst[e_dstl].astype(np.int32)
        e_col = (tile_col0[e_tile] + e_k).astype(np.int32)

        # slot-level source matrix (-1 = pad), then quads + per-sub dedup
        S = np.full((128, w_total), -1, np.int32)
        S[e_p, e_col] = e_src
        idxq = np.zeros((128, wq_total), np.int32)
        qrows_l, qnodes_l = [], []
        for s in range(NSUB):
            qsel = [
                (qcol0, Wq)
                for (ss, qcol0, Wq, parts) in calls
                if ss == s
            ]
            cols = np.concatenate(
                [np.arange(q0, q0 + Wq) for (q0, Wq) in qsel]
            )
            quads = S[:, (4 * cols[:, None] + np.arange(4)).reshape(-1)]
            quads = quads.reshape(128, len(cols), 4)
            flat = np.ascontiguousarray(quads.reshape(-1, 4))
            u, inv = np.unique(flat.view("V16").ravel(), return_inverse=True)
            nu = len(u)
            rsub_max = max(rsub_max, nu)
            uq = u.view(np.int32).reshape(-1, 4)
            idxq[:, cols] = inv.reshape(128, len(cols))
            qrows_l.append(uq)
        percore.append((perm, idxq, qrows_l, (e_p, e_col, e_src)))

    rsubq = -(-int(rsub_max) // 128) * 128
    for c in range(N_CORES):
        perm, idxq, qrows_l, edges = percore[c]
        idx16 = np.zeros((128, 8 * wq_total), np.int16)
        for (s, qcol0, Wq, parts) in calls:
            flat = idxq[:, qcol0 : qcol0 + Wq].T.ravel()
            idx16[:, 8 * qcol0 : 8 * (qcol0 + Wq)] = np.tile(
                flat.reshape(-1, 16).T, (8, 1)
            ).astype(np.int16)
        rows = np.concatenate(
            [s * rsubq + np.arange(len(qrows_l[s])) for s in range(NSUB)]
        )
        nodes = np.concatenate(qrows_l, axis=0)
        perms.append(perm)
        idx16s.append(idx16)
        qrowss.append((rows, nodes))
        edgess.append(edges)

    return Schedule(
        n_tiles,
        w_total,
        wq_total,
        rsubq,
        tiles,
        calls,
        perms,
        idx16s,
        qrowss,
        edgess,
    )


# ---------------------------------------------------------------- prog A
def build_progA(n_loc=DPC, in_c=IN_C, f=F):
    nc = bacc.Bacc("TRN2", target_bir_lowering=False, debug=False, num_devices=N_CORES)
    xT = nc.dram_tensor("xT", [in_c, n_loc], F16, kind="ExternalInput").ap()
    W = nc.dram_tensor("W", [in_c, f], F16, kind="ExternalInput").ap()
    bW = nc.dram_tensor("bW", [f, 1], F32, kind="ExternalInput").ap()
    WA = nc.dram_tensor("WA", [in_c, 2], F16, kind="ExternalInput").ap()
    bA2 = nc.dram_tensor("bA2", [2, 1], F32, kind="ExternalInput").ap()
    whT = nc.dram_tensor("whT", [f, n_loc], F32, kind="ExternalOutput").ap()
    s = nc.dram_tensor("s", [2, n_loc], F32, kind="ExternalOutput").ap()

    with tile.TileContext(nc) as tc:
        with tc.tile_pool(name="sb", bufs=1) as pool, tc.tile_pool(
            name="ps", bufs=4, space="PSUM"
        ) as pps, tc.tile_pool(name="sb2", bufs=3) as pool2:
            W_sb = pool.tile([in_c, f], F16)
            nc.sync.dma_start(out=W_sb[:], in_=W[:, :])
            bW_sb = pool.tile([f, 1], F32)
            nc.sync.dma_start(out=bW_sb[:], in_=bW[:, :])
            WA_sb = pool.tile([in_c, 2], F16)
            nc.sync.dma_start(out=WA_sb[:], in_=WA[:, :])
            bA2_sb = pool.tile([2, 1], F32)
            nc.sync.dma_start(out=bA2_sb[:], in_=bA2[:, :])
            xT_sb = pool.tile([in_c, n_loc], F16)
            XCH = 3125
            for x0 in range(0, n_loc, XCH):
                xc = min(XCH, n_loc - x0)
                nc.sync.dma_start(
                    out=xT_sb[:, x0 : x0 + xc], in_=xT[:, x0 : x0 + xc]
                )

            CH = 512
            GRP = 4  # store in 2048-column groups
            wh_g = None
            s_g = None
            for c0 in range(0, n_loc, CH):
                ch = min(CH, n_loc - c0)
                gi = (c0 // CH) % GRP
                if gi == 0:
                    wh_g = pool2.tile([f, GRP * CH], F32, tag="whg")
                    s_g = pool2.tile([2, GRP * CH], F32, tag="sg")
                ps_w = pps.tile([f, CH], F32, space="PSUM")
                nc.tensor.matmul(
                    out=ps_w[:, :ch],
                    lhsT=W_sb[:],
                    rhs=xT_sb[:, c0 : c0 + ch],
                    start=True,
                    stop=True,
                )
                nc.scalar.activation(
                    out=wh_g[:, gi * CH : gi * CH + ch],
                    in_=ps_w[:, :ch],
                    func=mybir.ActivationFunctionType.Identity,
                    bias=bW_sb[:],
                )
                ps_s = pps.tile([2, CH], F32, space="PSUM")
                nc.tensor.matmul(
                    out=ps_s[:, :ch],
                    lhsT=WA_sb[:],
                    rhs=xT_sb[:, c0 : c0 + ch],
                    start=True,
                    stop=True,
                )
                nc.vector.tensor_scalar(
                    out=s_g[:, gi * CH : gi * CH + ch],
                    in0=ps_s[:, :ch],
                    scalar1=bA2_sb[:, 0:1],
                    scalar2=None,
                    op0=mybir.AluOpType.add,
                )
                if gi == GRP - 1 or c0 + ch >= n_loc:
                    g0 = (c0 // CH // GRP) * GRP * CH
                    gl = c0 + ch - g0
                    nc.sync.dma_start(
                        out=whT[:, g0 : g0 + gl], in_=wh_g[:, :gl]
                    )
                    nc.sync.dma_start(out=s[:, g0 : g0 + gl], in_=s_g[:, :gl])
    nc.compile()
    return nc


# ---------------------------------------------------------------- prog B
def build_progB(sched: Schedule, f=F):
    NT = sched.n_tiles
    WTOT = sched.w_total
    WQTOT = sched.wq_total
    RSUBQ = sched.rsubq
    nc = bacc.Bacc("TRN2", target_bir_lowering=False, debug=False, num_devices=N_CORES)
    tableq = nc.dram_tensor(
        "tableq", [NSUB * RSUBQ, 2 * f], F32, kind="ExternalInput"
    ).ap()
    idx_d = nc.dram_tensor("idx", [128, 8 * WQTOT], I16, kind="ExternalInput").ap()
    sj_d = nc.dram_tensor("sj", [128, WTOT], F32, kind="ExternalInput").ap()
    si_d = nc.dram_tensor("si", [128, NT], F32, kind="ExternalInput").ap()
    # packed small consts: cols 0-63 Wn, 64 bWn, 65-66 As
    wp_d = nc.dram_tensor("wpack", [f, f + 4], F32, kind="ExternalInput").ap()
    whnT = nc.dram_tensor("whnT", [f, NT * 128], F32, kind="ExternalOutput").ap()
    sn = nc.dram_tensor("sn", [2, NT * 128], F32, kind="ExternalOutput").ap()

    X = mybir.AxisListType.X
    AF = mybir.ActivationFunctionType
    OP = mybir.AluOpType

    def v(ap, dims, off=0):
        return dataclasses.replace(
            ap,
            ap=[list(ap.ap[0])] + [list(d) for d in dims],
            offset=ap.offset + off,
        )

    nq = min(4, nc.num_swdge_queues)
    NTCMAX = max(ntc for (_, _, _, parts) in sched.calls for (_, _, ntc, _) in parts)

    with tile.TileContext(nc) as tc:
        with tc.tile_pool(name="const", bufs=1) as pc, tc.tile_pool(
            name="gat", bufs=3
        ) as pg, tc.tile_pool(name="exw", bufs=3) as px, tc.tile_pool(
            name="work", bufs=3
        ) as pw, tc.tile_pool(name="ht", bufs=2) as ph, tc.tile_pool(
            name="ps", bufs=2, space="PSUM"
        ) as pps, tc.tile_pool(name="ep", bufs=2) as pep:
            si_sb = pc.tile([128, NT], F32)
            nc.sync.dma_start(out=si_sb[:], in_=si_d[:, :])
            sj_sb = pc.tile([128, WTOT], F32)
            idx_sb = pc.tile([128, 8 * WQTOT], I16)
            wp_sb = pc.tile([f, f + 4], F32)
            Wn_sb = wp_sb[:, :f]
            bWn_sb = wp_sb[:, f : f + 1]
            As_sb = wp_sb[:, f + 1 : f + 3]
            ident = pc.tile([128, 128], F16)

            def emit_consts():
                # deferred past the first call's gather so the startup HWDGE
                # FIFO isn't serialized ahead of it
                nc.sync.dma_start(out=wp_sb[:], in_=wp_d[:, :])
                make_identity(nc, ident[:])

            ps_ch = None

            def flush_chunk(ck, ntl):
                cols = ntl * 128
                hTL = pep.tile([f, CHT * 128], F32, tag="hTL")
                nc.scalar.activation(
                    out=hTL[:, :cols],
                    in_=ps_ch[:, :cols],
                    func=AF.Identity,
                )
                ps_w = pps.tile([f, CHT * 128], F32, tag="psw", space="PSUM")
                nc.tensor.matmul(
                    out=ps_w[:, :cols],
                    lhsT=Wn_sb[:],
                    rhs=hTL[:, :cols],
                    start=True,
                    stop=True,
                )
                whn_sb = pep.tile([f, CHT * 128], F32, tag="whn")
                nc.scalar.activation(
                    out=whn_sb[:, :cols],
                    in_=ps_w[:, :cols],
                    func=AF.Identity,
                    bias=bWn_sb[:],
                )
                nc.sync.dma_start(
                    out=whnT[:, ck * CHT * 128 : ck * CHT * 128 + cols],
                    in_=whn_sb[:, :cols],
                )
                ps_s = pps.tile([2, CHT * 128], F32, tag="pss", space="PSUM")
                nc.tensor.matmul(
                    out=ps_s[:, :cols],
                    lhsT=As_sb,
                    rhs=whn_sb[:, :cols],
                    start=True,
                    stop=True,
                )
                s_sb = pep.tile([2, CHT * 128], F32, tag="ssb")
                nc.scalar.activation(
                    out=s_sb[:, :cols], in_=ps_s[:, :cols], func=AF.Identity
                )
                nc.sync.dma_start(
                    out=sn[:, ck * CHT * 128 : ck * CHT * 128 + cols],
                    in_=s_sb[:, :cols],
                )

            gq = 0

            def stage1(ci):
                nonlocal gq
                s, qcol0, Wq, parts = sched.calls[ci]
                # per-call slices of the idx / sj constants (shorter ramp);
                # sj first: the DVE's epre only needs sj+si, not the gather
                nc.sync.dma_start(
                    out=sj_sb[:, 4 * qcol0 : 4 * (qcol0 + Wq)],
                    in_=sj_d[:, 4 * qcol0 : 4 * (qcol0 + Wq)],
                )
                nc.sync.dma_start(
                    out=idx_sb[:, 8 * qcol0 : 8 * (qcol0 + Wq)],
                    in_=idx_d[:, 8 * qcol0 : 8 * (qcol0 + Wq)],
                )
                gbuf = pg.tile([128, WQMAX * 2 * f], F32, tag="gbuf")
                # hw limit: <=1024 indices per dma_gather -> <=8 quad columns
                for j0 in range(0, Wq, 8):
                    jw = min(8, Wq - j0)
                    nc.gpsimd.dma_gather(
                        out_ap=v(
                            gbuf[:], [(2 * f, jw), (1, 2 * f)], off=j0 * 2 * f
                        ),
                        in_ap=tableq[s * RSUBQ : (s + 1) * RSUBQ, :],
                        idxs_ap=idx_sb[:, 8 * (qcol0 + j0) : 8 * (qcol0 + j0 + jw)],
                        num_idxs=128 * jw,
                        num_idxs_reg=128 * jw,
                        elem_size=2 * f,
                        queue_num=gq % nq,
                    )
                    gq += 1
                gbuf16 = gbuf[:].bitcast(F16)  # slot i feats at f16 cols [64i,+64)
                exw = px.tile([128, WQMAX * 4 * f], F16, tag="exw")

                # attention logits + exp for every part
                for (t0, D, ntc, lc) in parts:
                    Wr = D * ntc
                    ls = lc  # slot offset within call
                    sc = 4 * qcol0 + lc  # global slot column
                    # e_pre = sj + si'  (si' = si + bA; sj = NEG_BIG at pads)
                    epre = pw.tile([128, 4 * WQMAX], F32, tag="epre")
                    nc.vector.tensor_tensor(
                        out=v(epre[:], [(D, ntc), (1, D)]),
                        in0=v(sj_sb[:], [(D, ntc), (1, D)], off=sc),
                        in1=si_sb[:, t0 : t0 + ntc].to_broadcast([128, ntc, D]),
                        op=OP.add,
                    )
                    # e = leaky_relu(e_pre)  (DVE: alpha*x then max)
                    e1 = pw.tile([128, 4 * WQMAX], F32, tag="e1")
                    nc.vector.tensor_scalar(
                        out=e1[:, :Wr],
                        in0=epre[:, :Wr],
                        scalar1=ALPHA,
                        scalar2=None,
                        op0=OP.mult,
                    )
                    nc.vector.tensor_tensor(
                        out=e1[:, :Wr], in0=e1[:, :Wr], in1=epre[:, :Wr], op=OP.max
                    )
                    # segment softmax over the slot axis
                    m = pw.tile([128, NTCMAX], F32, tag="m")
                    nc.vector.tensor_reduce(
                        out=m[:, :ntc],
                        in_=v(e1[:], [(D, ntc), (1, D)]),
                        axis=X,
                        op=OP.max,
                    )
                    nc.vector.tensor_tensor(
                        out=v(e1[:], [(D, ntc), (1, D)]),
                        in0=v(e1[:], [(D, ntc), (1, D)]),
                        in1=m[:, :ntc].to_broadcast([128, ntc, D]),
                        op=OP.subtract,
                    )
                    # exp, broadcast-expanded across the feature axis (fp16)
                    nc.scalar.activation(
                        out=v(exw[:], [(f * D, ntc), (f, D), (1, f)], off=ls * f),
                        in_=v(e1[:], [(D, ntc), (1, D), (0, f)]),
                        func=AF.Exp,
                    )
                return gbuf16, exw

            def stage2(ci, gbuf16, exw):
                nonlocal ps_ch
                s, qcol0, Wq, parts = sched.calls[ci]
                # denominator, weighted message sum, epilogue
                for (t0, D, ntc, lc) in parts:
                    Wr = D * ntc
                    ls = lc
                    den = pw.tile([128, NTCMAX], F32, tag="den")
                    nc.vector.tensor_reduce(
                        out=den[:, :ntc],
                        in_=v(exw[:], [(f * D, ntc), (f, D)], off=ls * f),
                        axis=X,
                        op=OP.add,
                    )
                    rden = pw.tile([128, NTCMAX], F32, tag="rden")
                    nc.vector.reciprocal(out=rden[:, :ntc], in_=den[:, :ntc])
                    # weighted messages: exw *= Wh16 (in place, 2x fp16)
                    nc.vector.tensor_tensor(
                        out=v(exw[:], [(f * D, ntc), (f, D), (1, f)], off=ls * f),
                        in0=v(gbuf16, [(f * D, ntc), (f, D), (1, f)], off=ls * f),
                        in1=v(exw[:], [(f * D, ntc), (f, D), (1, f)], off=ls * f),
                        op=OP.mult,
                    )
                    # halving-tree sum over slots -> slot 0 of each tile
                    cur = D
                    while cur > 1:
                        if cur % 2:
                            nc.vector.tensor_tensor(
                                out=v(exw[:], [(f * D, ntc), (1, f)], off=ls * f),
                                in0=v(exw[:], [(f * D, ntc), (1, f)], off=ls * f),
                                in1=v(
                                    exw[:],
                                    [(f * D, ntc), (1, f)],
                                    off=(ls + cur - 1) * f,
                                ),
                                op=OP.add,
                            )
                            cur -= 1
                        h = cur // 2
                        nc.vector.tensor_tensor(
                            out=v(exw[:], [(f * D, ntc), (f, h), (1, f)], off=ls * f),
                            in0=v(exw[:], [(f * D, ntc), (f, h), (1, f)], off=ls * f),
                            in1=v(
                                exw[:],
                                [(f * D, ntc), (f, h), (1, f)],
                                off=(ls + h) * f,
                            ),
                            op=OP.add,
                        )
                        cur = h
                    # normalize + leaky (dst-major, fp16), then transpose
                    ht = ph.tile([128, NTCMAX * f], F16, tag="ht")
                    nc.vector.tensor_tensor(
                        out=v(ht[:], [(f, ntc), (1, f)]),
                        in0=v(exw[:], [(f * D, ntc), (1, f)], off=ls * f),
                        in1=rden[:, :ntc].to_broadcast([128, ntc, f]),
                        op=OP.mult,
                    )
                    ht2 = ph.tile([128, NTCMAX * f], F16, tag="ht2")
                    nc.vector.tensor_scalar(
                        out=ht2[:, : ntc * f],
                        in0=ht[:, : ntc * f],
                        scalar1=ALPHA,
                        scalar2=None,
                        op0=OP.mult,
                    )
                    nc.vector.tensor_tensor(
                        out=ht[:, : ntc * f],
                        in0=ht[:, : ntc * f],
                        in1=ht2[:, : ntc * f],
                        op=OP.max,
                    )
                    for tl in range(ntc):
                        t = t0 + tl
                        j = t % CHT
                        if j == 0:
                            ps_ch = pps.tile(
                                [f, CHT * 128], F16, tag="psch", space="PSUM"
                            )
                        nc.tensor.transpose(
                            out=ps_ch[:, j * 128 : (j + 1) * 128],
                            in_=ht[:, tl * f : (tl + 1) * f],
                            identity=ident[:],
                        )
                        if j == CHT - 1 or t == NT - 1:
                            flush_chunk(t // CHT, j + 1)

            # software pipeline: stage1 of call N+1 is emitted before stage2
            # of call N so the DVE never stalls on Act's exp at call
            # boundaries
            prev = None
            for ci in range(len(sched.calls)):
                ctx = stage1(ci)
                if ci == 0:
                    emit_consts()
                if prev is not None:
                    stage2(prev[0], *prev[1])
                prev = (ci, ctx)
            stage2(prev[0], *prev[1])
    nc.compile()
    return nc


# ---------------------------------------------------------------- driver
_cache = {}
TRACE = False
LAST_HW_NS = []
LAST_RESULTS = []


def _run(nc, in_maps, cores):
    res = bass_utils.run_bass_kernel_spmd(nc, in_maps, core_ids=cores, trace=TRACE)
    if TRACE:
        LAST_RESULTS.append(res)
        if res.exec_time_ns:
            LAST_HW_NS.append(res.exec_time_ns)
    return res


def _get_schedule(edge_index):
    fp = hashlib.sha1(np.ascontiguousarray(edge_index)).hexdigest()
    key = ("sched", fp)
    if key not in _cache:
        _cache[key] = build_schedule(edge_index)
    return _cache[key]


def _pack_table(sched, c, wh16, sj_full):
    rows, nodes = sched.qrows[c]
    table = np.zeros((NSUB * sched.rsubq, 2 * F), np.float32)
    blk = wh16[np.clip(nodes, 0, N_NODES - 1)]
    blk[nodes < 0] = 0
    table[rows] = blk.reshape(len(rows), 4 * F).view(np.float32)
    return table


def kernel(x, edge_index, W1, bW1, A1, bA1, W2, bW2, A2, bA2, Wfc, bfc):
    x = np.asarray(x, dtype=np.float32)
    edge_index = np.asarray(edge_index)
    W1 = np.asarray(W1, np.float32)
    bW1 = np.asarray(bW1, np.float32)
    A1 = np.asarray(A1, np.float32)
    bA1 = np.asarray(bA1, np.float32)
    W2 = np.asarray(W2, np.float32)
    bW2 = np.asarray(bW2, np.float32)
    A2 = np.asarray(A2, np.float32)
    bA2 = np.asarray(bA2, np.float32)
    Wfc = np.asarray(Wfc, np.float32)
    bfc = np.asarray(bfc, np.float32)

    sched = _get_schedule(edge_index)
    cores = list(range(N_CORES))

    if "A" not in _cache:
        _cache["A"] = build_progA()
    ncA = _cache["A"]
    inA = []
    x16 = x.astype(np.float16)
    A1cat = np.concatenate([A1[:F], A1[F:]], axis=1)  # [64, 2]
    WA = (W1 @ A1cat).astype(np.float16)  # [128, 2]
    bA2v = (bW1 @ A1cat).reshape(2, 1).astype(np.float32)
    for c in cores:
        xT = np.ascontiguousarray(x16[c * DPC : (c + 1) * DPC].T)
        inA.append(
            {
                "xT": xT,
                "W": W1.astype(np.float16),
                "bW": bW1.reshape(F, 1),
                "WA": WA,
                "bA2": bA2v,
            }
        )
    LAST_HW_NS.clear()
    LAST_RESULTS.clear()
    resA = _run(ncA, inA, cores)
    wh = np.concatenate([resA.results[c]["whT"].T for c in cores], axis=0)
    s_all = np.concatenate([resA.results[c]["s"] for c in cores], axis=1)
    si_full, sj_full = s_all[0], s_all[1]

    key = ("B", sched.n_tiles, sched.wq_total, sched.rsubq, tuple(sched.tiles))
    if key not in _cache:
        _cache[key] = build_progB(sched)
    ncB = _cache[key]

    def launch_B(wh_full, si_f, sj_f, bA, Wn, bWn, An):
        bA0 = np.float32(bA.reshape(-1)[0])
        wh16 = wh_full.astype(np.float16)
        wpack = np.zeros((F, F + 4), np.float32)
        wpack[:, :F] = Wn
        wpack[:, F] = bWn.reshape(F)
        wpack[:, F + 1 : F + 3] = An
        inB = []
        for c in cores:
            perm = sched.perms[c]
            real = perm >= 0
            gids = c * DPC + perm[real]
            tmp = np.zeros(sched.n_tiles * 128, np.float32)
            tmp[real] = si_f[gids] + bA0
            si_arr = np.ascontiguousarray(tmp.reshape(sched.n_tiles, 128).T)
            e_p, e_col, e_src = sched.edges[c]
            sj_arr = np.full((128, sched.w_total), np.float32(NEG_BIG))
            sj_arr[e_p, e_col] = sj_f[e_src]
            inB.append(
                {
                    "tableq": _pack_table(sched, c, wh16, sj_f),
                    "idx": sched.idx16[c],
                    "sj": sj_arr,
                    "si": si_arr,
                    "wpack": wpack,
                }
            )
        res = _run(ncB, inB, cores)
        whn = np.zeros((N_NODES, F), np.float32)
        sn_i = np.zeros(N_NODES, np.float32)
        sn_j = np.zeros(N_NODES, np.float32)
        for c in cores:
            perm = sched.perms[c]
            real = perm >= 0
            gids = c * DPC + perm[real]
            whn[gids] = res.results[c]["whnT"].T[real]
            sn_c = res.results[c]["sn"]
            sn_i[gids] = sn_c[0][real]
            sn_j[gids] = sn_c[1][real]
        return whn, sn_i, sn_j

    As2 = np.ascontiguousarray(np.concatenate([A2[:F], A2[F:]], axis=1))
    wh2, si2, sj2 = launch_B(wh, si_full, sj_full, bA1, W2, bW2, As2)
    out, _, _ = launch_B(wh2, si2, sj2, bA2, Wfc, bfc, np.zeros((F, 2), np.float32))
    return out.astype(np.float32)


# revision 61
# speedup vs baseline: 1.0417x; 1.0051x over previous
"""GAT (2-layer) on 8 NeuronCores — Bass/Tile kernel.

Strategy (dst-sharded graph parallel):
  - Each core owns 12500 destination nodes, split into 6 sub-shards
    (round-robin over the degree-sorted order) so each sub-shard's quad
    table stays within dma_gather's int16 index range.
  - Slot layout: degree-sorted 128-dst tiles, per-tile slot capacity D from
    a multiple-of-4 grid. Slots are grouped 4-at-a-time into "quads"; the
    halo table holds one 512B row per distinct quad (4 x 64 fp16 features),
    so each gather descriptor moves 512B (no sub-512B DMA penalty).
  - Launch A: per-core Wh1^T = (x W1 + b)^T + attention scalars s_i/s_j.
  - Host between launches: packs quad tables from device-computed Wh
    (fp16), expands s_j per slot (f32, with -1e30 at pad slots, which
    doubles as the softmax pad mask), folds bA into s_i. Index-only work
    plus value repacking; all model FLOPs run on device.
  - Launch B (x2, one per GAT layer): wide quad dma_gathers, segment
    softmax over the slot axis (leaky-relu on DVE, exp broadcast-expanded
    to fp16 on Act), 2x-rate fp16 multiply + in-place halving-tree slot
    sum, alpha-normalize + leaky, per-tile PE transpose into shared PSUM
    chunks, epilogue matmul with the next layer's weights.
"""

import bisect
import dataclasses
import hashlib
import numpy as np

import concourse.bacc as bacc
import concourse.tile as tile
from concourse import bass, mybir, bass_utils
from concourse.masks import make_identity

F32 = mybir.dt.float32
F16 = mybir.dt.float16
I16 = mybir.dt.int16

N_NODES = 100000
N_CORES = 8
DPC = N_NODES // N_CORES
F = 64
IN_C = 128
NSUB = 6
WQMAX = 32  # quad columns per gather call (128 slots)
SMAX = 4 * WQMAX  # slot columns per gather call
GRID = [4, 8, 12, 16, 20, 24, 28, 32, 36, 40, 44, 48, 56, 64, 80, 96, 128]
CHT = 4  # tiles per epilogue chunk (512 dsts; fp32 matmul N<=512)
NEG_BIG = -1.0e30
ALPHA = 0.2


@dataclasses.dataclass
class Schedule:
    n_tiles: int
    w_total: int  # slot columns
    wq_total: int  # quad columns
    rsubq: int  # quad-table rows per sub-shard
    tiles: list  # per global tile: (sub, D)
    calls: list  # (sub, qcol0, Wq, parts) ; parts: [(tile0, D, ntc, lq)]
    perms: list  # per core: int64 [n_tiles*128], local dst or -1
    idx16: list  # per core: int16 [128, 8*wq_total]
    qrows: list  # per core: (row_ids, nodes[R,4] int32 with -1 pads)
    edges: list  # per core: (e_p, e_col, e_src) for sj_slot expansion


def _grid_up(x):
    return GRID[bisect.bisect_left(GRID, max(1, int(x)))]


def build_schedule(edge_index: np.ndarray) -> Schedule:
    src = np.asarray(edge_index[0], dtype=np.int64)
    dst = np.asarray(edge_index[1], dtype=np.int64)
    E = src.shape[0]
    order = np.argsort(dst, kind="stable")
    src_s = src[order]
    dst_s = dst[order]
    deg_all = np.bincount(dst, minlength=N_NODES)
    starts_all = np.concatenate([[0], np.cumsum(deg_all)])
    k_s = np.arange(E) - starts_all[dst_s]

    # ascending-degree round robin; the LAST sub-shard is reversed
    # (descending) so the program's final tiles are small and the epilogue
    # tail is short
    core_subs = []
    for c in range(N_CORES):
        deg = deg_all[c * DPC : (c + 1) * DPC]
        rank = np.argsort(deg, kind="stable")
        subs = [rank[s::NSUB] for s in range(NSUB)]
        subs[NSUB - 1] = subs[NSUB - 1][::-1]
        core_subs.append(subs)

    def _blockify_s(s, lst, nt):
        # partial tile holds the low-degree end: front-pad ascending subs,
        # back-pad the reversed (descending) last sub
        block = np.full(nt * 128, -1, np.int64)
        if s == NSUB - 1:
            block[: len(lst)] = lst
        else:
            block[nt * 128 - len(lst) :] = lst
        return block

    tiles = []
    sub_nt = []
    for s in range(NSUB):
        nt = max(-(-len(core_subs[c][s]) // 128) for c in range(N_CORES))
        sub_nt.append(nt)
        tmax = np.zeros(nt)
        for c in range(N_CORES):
            block = _blockify_s(s, core_subs[c][s], nt)
            d = np.where(
                block >= 0, deg_all[np.clip(c * DPC + block, 0, N_NODES - 1)], 0
            )
            tmax = np.maximum(tmax, d.reshape(nt, 128).max(1))
        for t in range(nt):
            tiles.append((s, _grid_up(tmax[t])))
    n_tiles = len(tiles)

    # runs of equal (sub, D) packed into gather calls of <= WQMAX quad cols
    runs = []
    i = 0
    while i < n_tiles:
        s, D = tiles[i]
        n = 1
        while i + n < n_tiles and tiles[i + n] == (s, D):
            n += 1
        runs.append((s, i, D, n))
        i += n
    # pack runs into calls by SLOT width; a call's quad width is its slot
    # width rounded up to a multiple of 4 (dead pad slots at the call end)
    calls = []
    cur_s, cur, cw = None, [], 0
    for (s, t0, D, n) in runs:
        rem_t0, rem_n = t0, n
        while rem_n:
            lim = 32 if len(calls) < 2 else SMAX  # small first calls: ramp
            lim = max(lim, D)
            if cur and (cur_s != s or cw + D > lim):
                calls.append((cur_s, 0, -(-cw // 4), cur))
                cur_s, cur, cw = None, [], 0
                continue
            take = min(rem_n, (lim - cw) // D)
            assert take > 0, (s, D, n, cw, lim)
            cur_s = s
            cur.append((rem_t0, D, take, cw))
            cw += D * take
            rem_t0 += take
            rem_n -= take
    if cur:
        calls.append((cur_s, 0, -(-cw // 4), cur))
    # split a small tail off the last call so the drain is short
    s_l, _, _, parts_l = calls[-1]
    tot_l = sum(D * ntc for (_, D, ntc, _) in parts_l)
    if tot_l > 32:
        target = tot_l - 16
        p1, p2, acc, w1, w2 = [], [], 0, 0, 0
        for (t0, D, ntc, lc) in parts_l:
            for tl in range(ntc):
                if acc < target:
                    if p1 and p1[-1][0] + p1[-1][2] == t0 + tl and p1[-1][1] == D:
                        p1[-1] = (p1[-1][0], D, p1[-1][2] + 1, p1[-1][3])
                    else:
                        p1.append((t0 + tl, D, 1, w1))
                    w1 += D
                else:
                    if p2 and p2[-1][0] + p2[-1][2] == t0 + tl and p2[-1][1] == D:
                        p2[-1] = (p2[-1][0], D, p2[-1][2] + 1, p2[-1][3])
                    else:
                        p2.append((t0 + tl, D, 1, w2))
                    w2 += D
                acc += D
        p1 = [tuple(x) for x in p1]
        p2 = [tuple(x) for x in p2]
        calls[-1] = (s_l, 0, -(-w1 // 4), p1)
        calls.append((s_l, 0, -(-w2 // 4), p2))
    qcol = 0
    for j, (s, _, Wq, parts) in enumerate(calls):
        calls[j] = (s, qcol, Wq, parts)
        qcol += Wq
    wq_total = qcol
    w_total = 4 * wq_total
    tile_col0 = np.zeros(n_tiles, np.int64)
    for (s, qcol0, Wq, parts) in calls:
        for (t0, D, ntc, lc) in parts:
            for tl in range(ntc):
                tile_col0[t0 + tl] = 4 * qcol0 + lc + tl * D
    sub_of_tile = np.array([s for (s, D) in tiles], np.int64)

    perms, idx16s, qrowss, edgess = [], [], [], []
    rsub_max = 0
    percore = []
    for c in range(N_CORES):
        perm = np.full(n_tiles * 128, -1, dtype=np.int64)
        ti = 0
        for s in range(NSUB):
            nt = sub_nt[s]
            perm[ti * 128 : (ti + nt) * 128] = _blockify_s(s, core_subs[c][s], nt)
            ti += nt

        real = perm >= 0
        pos_of_dst = np.empty(DPC, np.int64)
        pos_of_dst[perm[real]] = np.flatnonzero(real)
        gtile_of_dst = pos_of_dst // 128
        p_of_dst = pos_of_dst % 128

        lo, hi = starts_all[c * DPC], starts_all[(c + 1) * DPC]
        e_src = src_s[lo:hi].astype(np.int32)
        e_dstl = dst_s[lo:hi] - c * DPC
        e_k = k_s[lo:hi]
        e_tile = gtile_of_dst[e_dstl]
        e_p = p_of_dst[e_dstl].astype(np.int32)
        e_col = (tile_col0[e_tile] + e_k).astype(np.int32)

        # slot-level source matrix (-1 = pad), then quads + per-sub dedup
        S = np.full((128, w_total), -1, np.int32)
        S[e_p, e_col] = e_src
        idxq = np.zeros((128, wq_total), np.int32)
        qrows_l, qnodes_l = [], []
        for s in range(NSUB):
            qsel = [
                (qcol0, Wq)
                for (ss, qcol0, Wq, parts) in calls
                if ss == s
            ]
            cols = np.concatenate(
                [np.arange(q0, q0 + Wq) for (q0, Wq) in qsel]
            )
            quads = S[:, (4 * cols[:, None] + np.arange(4)).reshape(-1)]
            quads = quads.reshape(128, len(cols), 4)
            flat = np.ascontiguousarray(quads.reshape(-1, 4))
            u, inv = np.unique(flat.view("V16").ravel(), return_inverse=True)
            nu = len(u)
            rsub_max = max(rsub_max, nu)
            uq = u.view(np.int32).reshape(-1, 4)
            idxq[:, cols] = inv.reshape(128, len(cols))
            qrows_l.append(uq)
        percore.append((perm, idxq, qrows_l, (e_p, e_col, e_src)))

    rsubq = -(-int(rsub_max) // 128) * 128
    for c in range(N_CORES):
        perm, idxq, qrows_l, edges = percore[c]
        idx16 = np.zeros((128, 8 * wq_total), np.int16)
        for (s, qcol0, Wq, parts) in calls:
            flat = idxq[:, qcol0 : qcol0 + Wq].T.ravel()
            idx16[:, 8 * qcol0 : 8 * (qcol0 + Wq)] = np.tile(
                flat.reshape(-1, 16).T, (8, 1)
            ).astype(np.int16)
        rows = np.concatenate(
            [s * rsubq + np.arange(len(qrows_l[s])) for s in range(NSUB)]
        )
        nodes = np.concatenate(qrows_l, axis=0)
        perms.append(perm)
        idx16s.append(idx16)
        qrowss.append((rows, nodes))
        edgess.append(edges)

    return Schedule(
        n_tiles,
        w_total,
        wq_total,
        rsubq,
        tiles,
        calls,
        perms,
        idx16s,
        qrowss,
        edgess,
    )


# ---------------------------------------------------------------- prog A
def build_progA(n_loc=DPC, in_c=IN_C, f=F):
    nc = bacc.Bacc("TRN2", target_bir_lowering=False, debug=False, num_devices=N_CORES)
    xT = nc.dram_tensor("xT", [in_c, n_loc], F16, kind="ExternalInput").ap()
    W = nc.dram_tensor("W", [in_c, f], F16, kind="ExternalInput").ap()
    bW = nc.dram_tensor("bW", [f, 1], F32, kind="ExternalInput").ap()
    WA = nc.dram_tensor("WA", [in_c, 2], F16, kind="ExternalInput").ap()
    bA2 = nc.dram_tensor("bA2", [2, 1], F32, kind="ExternalInput").ap()
    whT = nc.dram_tensor("whT", [f, n_loc], F32, kind="ExternalOutput").ap()
    s = nc.dram_tensor("s", [2, n_loc], F32, kind="ExternalOutput").ap()

    with tile.TileContext(nc) as tc:
        with tc.tile_pool(name="sb", bufs=1) as pool, tc.tile_pool(
            name="ps", bufs=4, space="PSUM"
        ) as pps, tc.tile_pool(name="sb2", bufs=3) as pool2:
            W_sb = pool.tile([in_c, f], F16)
            nc.sync.dma_start(out=W_sb[:], in_=W[:, :])
            bW_sb = pool.tile([f, 1], F32)
            nc.sync.dma_start(out=bW_sb[:], in_=bW[:, :])
            WA_sb = pool.tile([in_c, 2], F16)
            nc.sync.dma_start(out=WA_sb[:], in_=WA[:, :])
            bA2_sb = pool.tile([2, 1], F32)
            nc.sync.dma_start(out=bA2_sb[:], in_=bA2[:, :])
            xT_sb = pool.tile([in_c, n_loc], F16)
            XCH = 3125
            for x0 in range(0, n_loc, XCH):
                xc = min(XCH, n_loc - x0)
                nc.sync.dma_start(
                    out=xT_sb[:, x0 : x0 + xc], in_=xT[:, x0 : x0 + xc]
                )

            CH = 512
            GRP = 4  # store in 2048-column groups
            wh_g = None
            s_g = None
            for c0 in range(0, n_loc, CH):
                ch = min(CH, n_loc - c0)
                gi = (c0 // CH) % GRP
                if gi == 0:
                    wh_g = pool2.tile([f, GRP * CH], F32, tag="whg")
                    s_g = pool2.tile([2, GRP * CH], F32, tag="sg")
                ps_w = pps.tile([f, CH], F32, space="PSUM")
                nc.tensor.matmul(
                    out=ps_w[:, :ch],
                    lhsT=W_sb[:],
                    rhs=xT_sb[:, c0 : c0 + ch],
                    start=True,
                    stop=True,
                )
                nc.scalar.activation(
                    out=wh_g[:, gi * CH : gi * CH + ch],
                    in_=ps_w[:, :ch],
                    func=mybir.ActivationFunctionType.Identity,
                    bias=bW_sb[:],
                )
                ps_s = pps.tile([2, CH], F32, space="PSUM")
                nc.tensor.matmul(
                    out=ps_s[:, :ch],
                    lhsT=WA_sb[:],
                    rhs=xT_sb[:, c0 : c0 + ch],
                    start=True,
                    stop=True,
                )
                nc.vector.tensor_scalar(
                    out=s_g[:, gi * CH : gi * CH + ch],
                    in0=ps_s[:, :ch],
                    scalar1=bA2_sb[:, 0:1],
                    scalar2=None,
                    op0=mybir.AluOpType.add,
                )
                if gi == GRP - 1 or c0 + ch >= n_loc:
                    g0 = (c0 // CH // GRP) * GRP * CH
                    gl = c0 + ch - g0
                    nc.sync.dma_start(
                        out=whT[:, g0 : g0 + gl], in_=wh_g[:, :gl]
                    )
                    nc.sync.dma_start(out=s[:, g0 : g0 + gl], in_=s_g[:, :gl])
    nc.compile()
    return nc


# ---------------------------------------------------------------- prog B
def build_progB(sched: Schedule, f=F):
    NT = sched.n_tiles
    WTOT = sched.w_total
    WQTOT = sched.wq_total
    RSUBQ = sched.rsubq
    nc = bacc.Bacc("TRN2", target_bir_lowering=False, debug=False, num_devices=N_CORES)
    tableq = nc.dram_tensor(
        "tableq", [NSUB * RSUBQ, 2 * f], F32, kind="ExternalInput"
    ).ap()
    idx_d = nc.dram_tensor("idx", [128, 8 * WQTOT], I16, kind="ExternalInput").ap()
    sj_d = nc.dram_tensor("sj", [128, WTOT], F32, kind="ExternalInput").ap()
    si_d = nc.dram_tensor("si", [128, NT], F32, kind="ExternalInput").ap()
    # packed small consts: cols 0-63 Wn, 64 bWn, 65-66 As
    wp_d = nc.dram_tensor("wpack", [f, f + 4], F32, kind="ExternalInput").ap()
    whnT = nc.dram_tensor("whnT", [f, NT * 128], F32, kind="ExternalOutput").ap()
    sn = nc.dram_tensor("sn", [2, NT * 128], F32, kind="ExternalOutput").ap()

    X = mybir.AxisListType.X
    AF = mybir.ActivationFunctionType
    OP = mybir.AluOpType

    def v(ap, dims, off=0):
        return dataclasses.replace(
            ap,
            ap=[list(ap.ap[0])] + [list(d) for d in dims],
            offset=ap.offset + off,
        )

    nq = min(4, nc.num_swdge_queues)
    NTCMAX = max(ntc for (_, _, _, parts) in sched.calls for (_, _, ntc, _) in parts)

    with tile.TileContext(nc) as tc:
        with tc.tile_pool(name="const", bufs=1) as pc, tc.tile_pool(
            name="gat", bufs=3
        ) as pg, tc.tile_pool(name="exw", bufs=3) as px, tc.tile_pool(
            name="work", bufs=3
        ) as pw, tc.tile_pool(name="ht", bufs=2) as ph, tc.tile_pool(
            name="ps", bufs=2, space="PSUM"
        ) as pps, tc.tile_pool(name="ep", bufs=3) as pep:
            si_sb = pc.tile([128, NT], F32)
            nc.sync.dma_start(out=si_sb[:], in_=si_d[:, :])
            sj_sb = pc.tile([128, WTOT], F32)
            idx_sb = pc.tile([128, 8 * WQTOT], I16)
            wp_sb = pc.tile([f, f + 4], F32)
            Wn_sb = wp_sb[:, :f]
            bWn_sb = wp_sb[:, f : f + 1]
            As_sb = wp_sb[:, f + 1 : f + 3]
            ident = pc.tile([128, 128], F16)

            def emit_consts():
                # deferred past the first call's gather so the startup HWDGE
                # FIFO isn't serialized ahead of it
                nc.sync.dma_start(out=wp_sb[:], in_=wp_d[:, :])
                make_identity(nc, ident[:])

            ps_ch = None

            def flush_chunk(ck, ntl):
                cols = ntl * 128
                hTL = pep.tile([f, CHT * 128], F32, tag="hTL")
                nc.scalar.activation(
                    out=hTL[:, :cols],
                    in_=ps_ch[:, :cols],
                    func=AF.Identity,
                )
                ps_w = pps.tile([f, CHT * 128], F32, tag="psw", space="PSUM")
                nc.tensor.matmul(
                    out=ps_w[:, :cols],
                    lhsT=Wn_sb[:],
                    rhs=hTL[:, :cols],
                    start=True,
                    stop=True,
                )
                whn_sb = pep.tile([f, CHT * 128], F32, tag="whn")
                nc.scalar.activation(
                    out=whn_sb[:, :cols],
                    in_=ps_w[:, :cols],
                    func=AF.Identity,
                    bias=bWn_sb[:],
                )
                nc.sync.dma_start(
                    out=whnT[:, ck * CHT * 128 : ck * CHT * 128 + cols],
                    in_=whn_sb[:, :cols],
                )
                ps_s = pps.tile([2, CHT * 128], F32, tag="pss", space="PSUM")
                nc.tensor.matmul(
                    out=ps_s[:, :cols],
                    lhsT=As_sb,
                    rhs=whn_sb[:, :cols],
                    start=True,
                    stop=True,
                )
                s_sb = pep.tile([2, CHT * 128], F32, tag="ssb")
                nc.scalar.activation(
                    out=s_sb[:, :cols], in_=ps_s[:, :cols], func=AF.Identity
                )
                nc.sync.dma_start(
                    out=sn[:, ck * CHT * 128 : ck * CHT * 128 + cols],
                    in_=s_sb[:, :cols],
                )

            gq = 0

            def stage1(ci):
                nonlocal gq
                s, qcol0, Wq, parts = sched.calls[ci]
                # per-call slices of the idx / sj constants (shorter ramp);
                # sj first: the DVE's epre only needs sj+si, not the gather
                nc.sync.dma_start(
                    out=sj_sb[:, 4 * qcol0 : 4 * (qcol0 + Wq)],
                    in_=sj_d[:, 4 * qcol0 : 4 * (qcol0 + Wq)],
                )
                nc.sync.dma_start(
                    out=idx_sb[:, 8 * qcol0 : 8 * (qcol0 + Wq)],
                    in_=idx_d[:, 8 * qcol0 : 8 * (qcol0 + Wq)],
                )
                gbuf = pg.tile([128, WQMAX * 2 * f], F32, tag="gbuf")
                # hw limit: <=1024 indices per dma_gather -> <=8 quad columns
                for j0 in range(0, Wq, 8):
                    jw = min(8, Wq - j0)
                    nc.gpsimd.dma_gather(
                        out_ap=v(
                            gbuf[:], [(2 * f, jw), (1, 2 * f)], off=j0 * 2 * f
                        ),
                        in_ap=tableq[s * RSUBQ : (s + 1) * RSUBQ, :],
                        idxs_ap=idx_sb[:, 8 * (qcol0 + j0) : 8 * (qcol0 + j0 + jw)],
                        num_idxs=128 * jw,
                        num_idxs_reg=128 * jw,
                        elem_size=2 * f,
                        queue_num=gq % nq,
                    )
                    gq += 1
                gbuf16 = gbuf[:].bitcast(F16)  # slot i feats at f16 cols [64i,+64)
                exw = px.tile([128, WQMAX * 4 * f], F16, tag="exw")

                # attention logits + exp for every part
                for (t0, D, ntc, lc) in parts:
                    Wr = D * ntc
                    ls = lc  # slot offset within call
                    sc = 4 * qcol0 + lc  # global slot column
                    # e_pre = sj + si'  (si' = si + bA; sj = NEG_BIG at pads)
                    epre = pw.tile([128, 4 * WQMAX], F32, tag="epre")
                    nc.vector.tensor_tensor(
                        out=v(epre[:], [(D, ntc), (1, D)]),
                        in0=v(sj_sb[:], [(D, ntc), (1, D)], off=sc),
                        in1=si_sb[:, t0 : t0 + ntc].to_broadcast([128, ntc, D]),
                        op=OP.add,
                    )
                    # e = leaky_relu(e_pre)  (DVE: alpha*x then max)
                    e1 = pw.tile([128, 4 * WQMAX], F32, tag="e1")
                    nc.vector.tensor_scalar(
                        out=e1[:, :Wr],
                        in0=epre[:, :Wr],
                        scalar1=ALPHA,
                        scalar2=None,
                        op0=OP.mult,
                    )
                    nc.vector.tensor_tensor(
                        out=e1[:, :Wr], in0=e1[:, :Wr], in1=epre[:, :Wr], op=OP.max
                    )
                    # segment softmax over the slot axis
                    m = pw.tile([128, NTCMAX], F32, tag="m")
                    nc.vector.tensor_reduce(
                        out=m[:, :ntc],
                        in_=v(e1[:], [(D, ntc), (1, D)]),
                        axis=X,
                        op=OP.max,
                    )
                    nc.vector.tensor_tensor(
                        out=v(e1[:], [(D, ntc), (1, D)]),
                        in0=v(e1[:], [(D, ntc), (1, D)]),
                        in1=m[:, :ntc].to_broadcast([128, ntc, D]),
                        op=OP.subtract,
                    )
                    # exp, broadcast-expanded across the feature axis (fp16)
                    nc.scalar.activation(
                        out=v(exw[:], [(f * D, ntc), (f, D), (1, f)], off=ls * f),
                        in_=v(e1[:], [(D, ntc), (1, D), (0, f)]),
                        func=AF.Exp,
                    )
                return gbuf16, exw

            def stage2(ci, gbuf16, exw):
                nonlocal ps_ch
                s, qcol0, Wq, parts = sched.calls[ci]
                # denominator, weighted message sum, epilogue
                for (t0, D, ntc, lc) in parts:
                    Wr = D * ntc
                    ls = lc
                    den = pw.tile([128, NTCMAX], F32, tag="den")
                    nc.vector.tensor_reduce(
                        out=den[:, :ntc],
                        in_=v(exw[:], [(f * D, ntc), (f, D)], off=ls * f),
                        axis=X,
                        op=OP.add,
                    )
                    rden = pw.tile([128, NTCMAX], F32, tag="rden")
                    nc.vector.reciprocal(out=rden[:, :ntc], in_=den[:, :ntc])
                    # weighted messages: exw *= Wh16 (in place, 2x fp16)
                    nc.vector.tensor_tensor(
                        out=v(exw[:], [(f * D, ntc), (f, D), (1, f)], off=ls * f),
                        in0=v(gbuf16, [(f * D, ntc), (f, D), (1, f)], off=ls * f),
                        in1=v(exw[:], [(f * D, ntc), (f, D), (1, f)], off=ls * f),
                        op=OP.mult,
                    )
                    # halving-tree sum over slots -> slot 0 of each tile
                    cur = D
                    while cur > 1:
                        if cur % 2:
                            nc.vector.tensor_tensor(
                                out=v(exw[:], [(f * D, ntc), (1, f)], off=ls * f),
                                in0=v(exw[:], [(f * D, ntc), (1, f)], off=ls * f),
                                in1=v(
                                    exw[:],
                                    [(f * D, ntc), (1, f)],
                                    off=(ls + cur - 1) * f,
                                ),
                                op=OP.add,
                            )
                            cur -= 1
                        h = cur // 2
                        nc.vector.tensor_tensor(
                            out=v(exw[:], [(f * D, ntc), (f, h), (1, f)], off=ls * f),
                            in0=v(exw[:], [(f * D, ntc), (f, h), (1, f)], off=ls * f),
                            in1=v(
                                exw[:],
                                [(f * D, ntc), (f, h), (1, f)],
                                off=(ls + h) * f,
                            ),
                            op=OP.add,
                        )
                        cur = h
                    # normalize + leaky (dst-major, fp16), then transpose
                    ht = ph.tile([128, NTCMAX * f], F16, tag="ht")
                    nc.vector.tensor_tensor(
                        out=v(ht[:], [(f, ntc), (1, f)]),
                        in0=v(exw[:], [(f * D, ntc), (1, f)], off=ls * f),
                        in1=rden[:, :ntc].to_broadcast([128, ntc, f]),
                        op=OP.mult,
                    )
                    ht2 = ph.tile([128, NTCMAX * f], F16, tag="ht2")
                    nc.vector.tensor_scalar(
                        out=ht2[:, : ntc * f],
                        in0=ht[:, : ntc * f],
                        scalar1=ALPHA,
                        scalar2=None,
                        op0=OP.mult,
                    )
                    nc.vector.tensor_tensor(
                        out=ht[:, : ntc * f],
                        in0=ht[:, : ntc * f],
                        in1=ht2[:, : ntc * f],
                        op=OP.max,
                    )
                    for tl in range(ntc):
                        t = t0 + tl
                        j = t % CHT
                        if j == 0:
                            ps_ch = pps.tile(
                                [f, CHT * 128], F16, tag="psch", space="PSUM"
                            )
                        nc.tensor.transpose(
                            out=ps_ch[:, j * 128 : (j + 1) * 128],
                            in_=ht[:, tl * f : (tl + 1) * f],
                            identity=ident[:],
                        )
                        if j == CHT - 1 or t == NT - 1:
                            flush_chunk(t // CHT, j + 1)

            # software pipeline: stage1 of call N+1 is emitted before stage2
            # of call N so the DVE never stalls on Act's exp at call
            # boundaries
            prev = None
            for ci in range(len(sched.calls)):
                ctx = stage1(ci)
                if ci == 0:
                    emit_consts()
                if prev is not None:
                    stage2(prev[0], *prev[1])
                prev = (ci, ctx)
            stage2(prev[0], *prev[1])
    nc.compile()
    return nc


# ---------------------------------------------------------------- driver
_cache = {}
TRACE = False
LAST_HW_NS = []
LAST_RESULTS = []


def _run(nc, in_maps, cores):
    res = bass_utils.run_bass_kernel_spmd(nc, in_maps, core_ids=cores, trace=TRACE)
    if TRACE:
        LAST_RESULTS.append(res)
        if res.exec_time_ns:
            LAST_HW_NS.append(res.exec_time_ns)
    return res


def _get_schedule(edge_index):
    fp = hashlib.sha1(np.ascontiguousarray(edge_index)).hexdigest()
    key = ("sched", fp)
    if key not in _cache:
        _cache[key] = build_schedule(edge_index)
    return _cache[key]


def _pack_table(sched, c, wh16, sj_full):
    rows, nodes = sched.qrows[c]
    table = np.zeros((NSUB * sched.rsubq, 2 * F), np.float32)
    blk = wh16[np.clip(nodes, 0, N_NODES - 1)]
    blk[nodes < 0] = 0
    table[rows] = blk.reshape(len(rows), 4 * F).view(np.float32)
    return table


def kernel(x, edge_index, W1, bW1, A1, bA1, W2, bW2, A2, bA2, Wfc, bfc):
    x = np.asarray(x, dtype=np.float32)
    edge_index = np.asarray(edge_index)
    W1 = np.asarray(W1, np.float32)
    bW1 = np.asarray(bW1, np.float32)
    A1 = np.asarray(A1, np.float32)
    bA1 = np.asarray(bA1, np.float32)
    W2 = np.asarray(W2, np.float32)
    bW2 = np.asarray(bW2, np.float32)
    A2 = np.asarray(A2, np.float32)
    bA2 = np.asarray(bA2, np.float32)
    Wfc = np.asarray(Wfc, np.float32)
    bfc = np.asarray(bfc, np.float32)

    sched = _get_schedule(edge_index)
    cores = list(range(N_CORES))

    if "A" not in _cache:
        _cache["A"] = build_progA()
    ncA = _cache["A"]
    inA = []
    x16 = x.astype(np.float16)
    A1cat = np.concatenate([A1[:F], A1[F:]], axis=1)  # [64, 2]
    WA = (W1 @ A1cat).astype(np.float16)  # [128, 2]
    bA2v = (bW1 @ A1cat).reshape(2, 1).astype(np.float32)
    for c in cores:
        xT = np.ascontiguousarray(x16[c * DPC : (c + 1) * DPC].T)
        inA.append(
            {
                "xT": xT,
                "W": W1.astype(np.float16),
                "bW": bW1.reshape(F, 1),
                "WA": WA,
                "bA2": bA2v,
            }
        )
    LAST_HW_NS.clear()
    LAST_RESULTS.clear()
    resA = _run(ncA, inA, cores)
    wh = np.concatenate([resA.results[c]["whT"].T for c in cores], axis=0)
    s_all = np.concatenate([resA.results[c]["s"] for c in cores], axis=1)
    si_full, sj_full = s_all[0], s_all[1]

    key = ("B", sched.n_tiles, sched.wq_total, sched.rsubq, tuple(sched.tiles))
    if key not in _cache:
        _cache[key] = build_progB(sched)
    ncB = _cache[key]

    def launch_B(wh_full, si_f, sj_f, bA, Wn, bWn, An):
        bA0 = np.float32(bA.reshape(-1)[0])
        wh16 = wh_full.astype(np.float16)
        wpack = np.zeros((F, F + 4), np.float32)
        wpack[:, :F] = Wn
        wpack[:, F] = bWn.reshape(F)
        wpack[:, F + 1 : F + 3] = An
        inB = []
        for c in cores:
            perm = sched.perms[c]
            real = perm >= 0
            gids = c * DPC + perm[real]
            tmp = np.zeros(sched.n_tiles * 128, np.float32)
            tmp[real] = si_f[gids] + bA0
            si_arr = np.ascontiguousarray(tmp.reshape(sched.n_tiles, 128).T)
            e_p, e_col, e_src = sched.edges[c]
            sj_arr = np.full((128, sched.w_total), np.float32(NEG_BIG))
            sj_arr[e_p, e_col] = sj_f[e_src]
            inB.append(
                {
                    "tableq": _pack_table(sched, c, wh16, sj_f),
                    "idx": sched.idx16[c],
                    "sj": sj_arr,
                    "si": si_arr,
                    "wpack": wpack,
                }
            )
        res = _run(ncB, inB, cores)
        whn = np.zeros((N_NODES, F), np.float32)
        sn_i = np.zeros(N_NODES, np.float32)
        sn_j = np.zeros(N_NODES, np.float32)
        for c in cores:
            perm = sched.perms[c]
            real = perm >= 0
            gids = c * DPC + perm[real]
            whn[gids] = res.results[c]["whnT"].T[real]
            sn_c = res.results[c]["sn"]
            sn_i[gids] = sn_c[0][real]
            sn_j[gids] = sn_c[1][real]
        return whn, sn_i, sn_j

    As2 = np.ascontiguousarray(np.concatenate([A2[:F], A2[F:]], axis=1))
    wh2, si2, sj2 = launch_B(wh, si_full, sj_full, bA1, W2, bW2, As2)
    out, _, _ = launch_B(wh2, si2, sj2, bA2, Wfc, bfc, np.zeros((F, 2), np.float32))
    return out.astype(np.float32)
